# revision 30
# baseline (speedup 1.0000x reference)
"""GCN (2-layer, GCNConv + log_softmax) on 8 Trainium2 NeuronCores.

Strategy (1D node partition, per sharding hint):
  - Nodes padded to N_PAD = 392*128 and assigned to 392 blocks of 128 by a
    host-side balancing permutation (snake-deal by in-degree).
  - Self-loop terms are NOT edge slots: per dst block they are the affine
    rows of the core's own h block times diag(dinv^2), folded into the PSUM
    accumulation as one extra matmul (saves ~6% of gather descriptors).
  - Remaining edges, sorted by (dst block, src-half), are packed per block
    into lo/hi gather index lists (dma_gather indices are int16, so each
    half-table gather uses indices relative to its half). Tile counts
    (Tlo | Thi) are uniform across cores/blocks, but the per-(core,block)
    REAL counts are loaded into a register at runtime (num_idxs_reg), so
    padding costs no descriptors; pads are -1 (skipped by the DGE).
  - On device per core:
      GEMM1: h = x_shard @ W1 (PE, bf16); x shard SBUF-resident.
      AllGather h -> full h table (bf16) in local HBM.
      Agg1 per dst block: two batched dma_gathers (lo/hi halves) fetch the
        edge messages h[src]; selector S[e,dst] = (seg[e]==dst)*norm[e]
        built on DVE in bf16 for a whole 7-block chunk at once; segment-sum
        via PE matmul accumulation into PSUM [hid, dst] (+ self-loop
        matmul); relu(agg+b1) on ACT; fused GEMM2 -> h2 block (bf16,
        padded to 128 cols); chunk-buffered store.
      AllGather h2 (bf16) -> full h2 table.
      Agg2 per dst block: same gathers of h2[src], same selector,
        accumulate [dst, cls] (+ self-loop); +b2; log_softmax batched per
        7-block chunk; chunk output DMA.
  - Host concatenates the 8 output shards and un-permutes.
"""

import math

import numpy as np
import ml_dtypes

P = 128
NCORES = 8
CHUNK = 7               # blocks per chunk (49 = 7*7)

# Full-problem constants (hardcoded per harness contract).
N_NODES = 50000
N_EDGES = 800000
F_IN = 512
HIDDEN = 128
N_CLASSES = 40

# Runtime-tunable knobs (test.py may override before calling kernel()).
TRACE = False
TRACE_KWARGS = {}

# A/B debug knobs (wrong results when set; timing only).
SKIP_GATHER = False
SKIP_SEL = False
SKIP_MM = False

# Trim gather counts to the per-block 8-core max (const num_idxs_reg).
# Measured SLOWER on HW (sub-128 num_idxs hits a slow ucode path) - keep off.
TRIM_COUNTS = False

LAST_RESULT = {}        # test.py introspection (exec time etc.)


# --------------------------------------------------------------------------
# CPU preprocessing
# --------------------------------------------------------------------------

def _balance_perm(deg, n_pad):
    """Assign node ids to n_pad slots so each 128-slot block has ~equal
    total degree. Returns perm: perm[new_slot] = old_node (or -1 for pad).
    """
    n = deg.shape[0]
    nblk = n_pad // P
    order = np.argsort(-deg, kind="stable")
    perm = np.full(n_pad, -1, dtype=np.int64)
    blk_fill = np.zeros(nblk, dtype=np.int64)
    pos = 0
    rnd = 0
    while pos < n:
        take = min(nblk, n - pos)
        blocks = np.arange(nblk) if rnd % 2 == 0 else np.arange(nblk)[::-1]
        blocks = blocks[:take]
        perm[blocks * P + blk_fill[blocks]] = order[pos:pos + take]
        blk_fill[blocks] += 1
        pos += take
        rnd += 1
    return perm


def _preprocess(edge_index, n_nodes, blocks_per_core):
    """Returns (idx16, counts, segs, norms, dinv2, Tlo, Thi, perm):
      idx16  [NCORES, n_pos] int16  gather index stream; per block Tlo*128
             lo slots then Thi*128 hi slots; hi indices relative to the hi
             half; pads are -1 (skipped via num_idxs_reg).
      counts [NCORES, 2*BPC] int32  real (lo, hi) slot counts per block
      segs   [NCORES, 128, BPC*T] f32 local dst row per slot
      norms  [NCORES, 128, BPC*T] f32 dinv[src]*dinv[dst] (0 for pads)
      dinv2  [NCORES, 128, BPC]  f32 dinv[node]^2 per (row, block)
    """
    nblk = NCORES * blocks_per_core
    n_pad = nblk * P
    half = n_pad // 2
    src = np.asarray(edge_index[0], dtype=np.int64)
    dst = np.asarray(edge_index[1], dtype=np.int64)

    deg = np.bincount(dst, minlength=n_nodes).astype(np.float32) + 1.0
    dinv = (1.0 / np.sqrt(deg)).astype(np.float32)

    perm = _balance_perm(deg, n_pad)
    inv = np.zeros(n_nodes, dtype=np.int64)
    valid = perm >= 0
    inv[perm[valid]] = np.nonzero(valid)[0]

    # --- lo/hi 2-coloring: rebalance which nodes sit in the lo half so
    # every dst block's hi-edge count fits in Thi=ceil((cnt-lo_target)/128)
    # tiles. Same-degree nodes are interchangeable between slots without
    # disturbing the per-block degree balance, so we recolor within degree
    # classes. Window: lo in [cnt-1024+margin, 1152-margin] -> Tlo=9, Thi=8.
    half = n_pad // 2
    blk_of_dst = inv[dst] // P
    nblk_l = nblk
    cntb = np.bincount(blk_of_dst, minlength=nblk_l).astype(np.int64)
    # per-node out-edge block lists (by src node id)
    o_e = np.argsort(src, kind="stable")
    es_n = src[o_e]
    eb_n = blk_of_dst[o_e]
    st = np.searchsorted(es_n, np.arange(n_nodes))
    en = np.searchsorted(es_n, np.arange(n_nodes) + 1)
    lo_target = np.maximum(cntb - 985, cntb // 2 - 500).astype(np.float64)
    lo_cnt_f = np.zeros(nblk_l, np.float64)
    want_lo = np.zeros(n_nodes, bool)
    rngc = np.random.default_rng(12345)
    # Per-degree-class lo-slot quotas (from the snake placement) so the
    # later reassignment never needs to flip a want.
    degv_q = deg.astype(np.int64)
    slot_is_lo = inv < half
    qmax = degv_q.max() + 1
    lo_quota = np.bincount(degv_q[slot_is_lo], minlength=qmax)
    hi_quota = np.bincount(degv_q[~slot_is_lo], minlength=qmax)
    for v in rngc.permutation(n_nodes):
        dv = degv_q[v]
        a, b_ = st[v], en[v]
        if lo_quota[dv] == 0:
            c = False
        elif hi_quota[dv] == 0:
            c = True
        elif a == b_:
            c = lo_quota[dv] >= hi_quota[dv]
        else:
            bs = eb_n[a:b_]
            gap = (lo_target[bs] - lo_cnt_f[bs]).sum()
            c = gap > 0
        if c:
            want_lo[v] = True
            lo_quota[dv] -= 1
            if a != b_:
                lo_cnt_f[eb_n[a:b_]] += 1.0
        else:
            hi_quota[dv] -= 1

    # Reassign slots within degree classes to honor want_lo where possible.
    degv = deg.astype(np.int64)  # degree class key per node
    slot_of = inv.copy()
    new_perm = perm.copy()
    for dv in np.unique(degv):
        members = np.nonzero(degv == dv)[0]
        slots = slot_of[members]
        slo = slots < half
        w = want_lo[members]
        n_lo_avail = int(slo.sum())
        lo_members = members[w]
        hi_members = members[~w]
        if len(lo_members) > n_lo_avail:
            # flip excess wants (arbitrary)
            flip = len(lo_members) - n_lo_avail
            hi_members = np.concatenate([hi_members, lo_members[-flip:]])
            lo_members = lo_members[:-flip]
        elif n_lo_avail > len(lo_members):
            take = n_lo_avail - len(lo_members)
            lo_members = np.concatenate([lo_members, hi_members[-take:]])
            hi_members = hi_members[:-take]
        lo_slots = slots[slo]
        hi_slots = slots[~slo]
        new_perm[lo_slots] = lo_members
        new_perm[hi_slots] = hi_members
    perm = new_perm
    inv = np.zeros(n_nodes, dtype=np.int64)
    valid = perm >= 0
    inv[perm[valid]] = np.nonzero(valid)[0]

    all_src = inv[src]
    all_dst = inv[dst]
    norm = (dinv[src] * dinv[dst]).astype(np.float32)

    # dinv^2 per slot (self-loop diagonal); zero for pad slots.
    dinv2_slot = np.zeros(n_pad, np.float32)
    dinv2_slot[valid] = (dinv * dinv)[perm[valid]]
    dinv2 = np.ascontiguousarray(
        dinv2_slot.reshape(NCORES, blocks_per_core, P).transpose(0, 2, 1)
    )

    # Sort by (dst block, src-half) so each block's lo edges precede its
    # hi edges.
    is_hi = (all_src >= half).astype(np.int64)
    key = (all_dst // P) * 2 + is_hi
    order = np.argsort(key, kind="stable")
    s_src = all_src[order]
    s_dst = all_dst[order]
    s_norm = norm[order]
    s_hi = is_hi[order]

    blk = s_dst // P
    seg = (s_dst % P).astype(np.float32)
    nlo = np.bincount(blk[s_hi == 0], minlength=nblk)
    nhi = np.bincount(blk[s_hi == 1], minlength=nblk)
    Tlo = max(1, int(math.ceil(nlo.max() / P)))
    Thi = max(1, int(math.ceil(nhi.max() / P)))
    T = Tlo + Thi

    nt = blocks_per_core * T
    n_pos = nt * P
    idx16 = np.zeros((NCORES, n_pos), np.int16)
    counts = np.zeros((NCORES, 2 * blocks_per_core), np.int32)
    segs = np.zeros((NCORES, P, nt), np.float32)
    norms = np.zeros((NCORES, P, nt), np.float32)

    cnt_all = np.bincount(blk, minlength=nblk)
    starts = np.concatenate([[0], np.cumsum(cnt_all)])
    for b in range(nblk):
        c, bl = divmod(b, blocks_per_core)
        lo, hi = int(starts[b]), int(starts[b + 1])
        n_lo = int(nlo[b])
        base = bl * T * P
        # lo edges -> slots [0, n_lo), hi edges -> slots [Tlo*128, ...)
        for (e0, e1, s0, rel, ci) in (
            (lo, lo + n_lo, 0, 0, 2 * bl),
            (lo + n_lo, hi, Tlo * P, half, 2 * bl + 1),
        ):
            n = e1 - e0
            if n == 0:
                # keep one harmless slot so num_idxs_reg >= 1
                idx16[c, base + s0] = 0
                counts[c, ci] = 1
                continue
            counts[c, ci] = n
            i = np.arange(n) + s0
            idx16[c, base + i] = (s_src[e0:e1] - rel).astype(np.int16)
            g = bl * T + i // P
            p = i % P
            segs[c, p, g] = seg[e0:e1]
            norms[c, p, g] = s_norm[e0:e1]
    # Host-built static selector table: sel[c, g, p, d] = (segs==d)*norms
    # laid out [P, nt*P] bf16 per core (partition-major rows).
    bf = ml_dtypes.bfloat16
    selt = np.zeros((NCORES, P, nt, P), np.float32)
    cc = np.arange(NCORES)[:, None, None]
    pp = np.arange(P)[None, :, None]
    gg = np.arange(nt)[None, None, :]
    np.put_along_axis(
        selt, segs.astype(np.int64)[..., None], norms[..., None], axis=3
    )
    # zero out pad slots explicitly (norm==0 already, but seg 0 collisions
    # with norm 0 write zeros anyway)
    sel_tab = selt.reshape(NCORES, P, nt * P).astype(bf)

    # diag table: diag[c, b, p, d] = (p==d) * dinv2[c, p, b]
    eye = np.eye(P, dtype=np.float32)
    diag_tab = (
        eye[None, None, :, :] * dinv2.transpose(0, 2, 1)[:, :, :, None]
    ).transpose(0, 2, 1, 3).reshape(NCORES, P, blocks_per_core * P).astype(bf)

    # Per-block valid-count constants: max over cores; pad idx=0 (harmless,
    # norm 0) up to the block max so every core gathers exactly blk_max slots.
    cl = counts[:, 0::2]
    chh = counts[:, 1::2]
    blk_max_lo = cl.max(axis=0)
    blk_max_hi = chh.max(axis=0)
    for c in range(NCORES):
        for bl in range(blocks_per_core):
            base = bl * T * P
            for (n, m, s0) in (
                (int(cl[c, bl]), int(blk_max_lo[bl]), 0),
                (int(chh[c, bl]), int(blk_max_hi[bl]), Tlo * P),
            ):
                pass
    # round counts up to multiples of 16 (idx stream granularity)
    blk_max_lo = (blk_max_lo + 15) // 16 * 16
    blk_max_hi = (blk_max_hi + 15) // 16 * 16
    if not TRIM_COUNTS:
        blk_max_lo = np.full(blocks_per_core, Tlo * P)
        blk_max_hi = np.full(blocks_per_core, Thi * P)
    return (idx16, counts, sel_tab, diag_tab, Tlo, Thi, perm,
            blk_max_lo.astype(np.int64), blk_max_hi.astype(np.int64))


# --------------------------------------------------------------------------
# Device program
# --------------------------------------------------------------------------

def _build_program(f_in, hidden, ncls_pad, blocks_per_core, Tlo, Thi,
                   blk_max_lo, blk_max_hi):
    import concourse.bacc as bacc
    import concourse.bass as bass
    import concourse.mybir as mybir
    import concourse.tile as tile

    dt = mybir.dt
    bf16 = dt.bfloat16
    f32 = dt.float32

    T = Tlo + Thi
    shard = blocks_per_core * P
    n_pad = NCORES * shard
    half = n_pad // 2
    nt = blocks_per_core * T
    n_pos = nt * P
    kt = f_in // P
    C = CHUNK if blocks_per_core % CHUNK == 0 else (
        blocks_per_core if blocks_per_core <= CHUNK else 1)
    nchunk = blocks_per_core // C
    assert nchunk * C == blocks_per_core

    nc = bacc.Bacc(
        "TRN2",
        target_bir_lowering=False,
        debug=False,
        enable_asserts=False,
        num_devices=NCORES,
    )

    # Kernel I/O
    xt_d = nc.dram_tensor("xt", [f_in, shard], bf16, kind="ExternalInput")
    w1_d = nc.dram_tensor("w1", [P, kt * hidden], bf16, kind="ExternalInput")
    b1_d = nc.dram_tensor("b1", [P, 1], f32, kind="ExternalInput")
    w2_d = nc.dram_tensor("w2", [hidden, ncls_pad], bf16, kind="ExternalInput")
    b2_d = nc.dram_tensor("b2t", [P, N_CLASSES], f32, kind="ExternalInput")
    idx_d = nc.dram_tensor("idx16", [P, n_pos // 16], dt.int16, kind="ExternalInput")
    sel_d = nc.dram_tensor("selt", [P, nt * P], bf16, kind="ExternalInput")
    diag_d = nc.dram_tensor("diagt", [P, blocks_per_core * P], bf16,
                            kind="ExternalInput")
    out_d = nc.dram_tensor("out", [shard, N_CLASSES], f32, kind="ExternalOutput")

    RG = [list(range(NCORES))]

    with tile.TileContext(nc) as tc:
        with (
            tc.tile_pool(name="const", bufs=1) as const,
            tc.tile_pool(name="dram", bufs=1, space="DRAM") as dram,
            tc.tile_pool(name="sb", bufs=3) as sb,
            tc.tile_pool(name="wide", bufs=2) as wide,
            tc.tile_pool(name="psum", bufs=2, space="PSUM") as psum,
        ):
            # Internal DRAM buffers
            h_ag_in = dram.tile([shard, hidden], bf16)
            h_full = dram.tile([n_pad, hidden], bf16, addr_space="Shared")
            h2_ag_in = dram.tile([shard, ncls_pad], bf16)
            h2_full = dram.tile([n_pad, ncls_pad], bf16, addr_space="Shared")

            # Constants into SBUF
            w1_sb = const.tile([P, kt * hidden], bf16)
            nc.sync.dma_start(out=w1_sb[:], in_=w1_d[:])
            b1_sb = const.tile([P, 1], f32)
            nc.sync.dma_start(out=b1_sb[:], in_=b1_d[:])
            w2_sb = const.tile([hidden, ncls_pad], bf16)
            nc.sync.dma_start(out=w2_sb[:], in_=w2_d[:])
            b2_sb = const.tile([P, N_CLASSES], f32)
            nc.sync.dma_start(out=b2_sb[:], in_=b2_d[:])
            idx_sb = const.tile([P, n_pos // 16], dt.int16)
            nc.sync.dma_start(out=idx_sb[:], in_=idx_d[:])

            # x shard SBUF-resident (kt slabs of the transposed x).
            xt_sb = const.tile([P, kt * shard], bf16)
            for k in range(kt):
                nc.sync.dma_start(
                    out=xt_sb[:, k * shard:(k + 1) * shard],
                    in_=xt_d[k * P:(k + 1) * P, :],
                )

            # Prime rotating gather buffers so skipped pad slots hold finite
            # stale data (never NaN canaries) before the first real use.
            for tag, width in (("msg", hidden), ("msg2", ncls_pad)):
                for _ in range(4):
                    m = sb.tile([P, T * width], bf16, tag=tag, bufs=4)
                    nc.vector.memset(m[:], 0)

            # ---------------- Phase 1: GEMM1 (h = x @ W1) ----------------
            for i in range(blocks_per_core):
                psum_h = psum.tile([P, hidden], f32, tag="psum_h")
                for k in range(kt):
                    nc.tensor.matmul(
                        out=psum_h[:],
                        lhsT=xt_sb[:, k * shard + i * P:k * shard + (i + 1) * P],
                        rhs=w1_sb[:, k * hidden:(k + 1) * hidden],
                        start=(k == 0),
                        stop=(k == kt - 1),
                    )
                h_t = sb.tile([P, hidden], bf16, tag="h_t")
                nc.vector.tensor_copy(out=h_t[:], in_=psum_h[:])
                nc.sync.dma_start(
                    out=h_ag_in[i * P:(i + 1) * P, :], in_=h_t[:]
                )

            # ---------------- AllGather h ----------------
            nc.gpsimd.collective_compute(
                "AllGather",
                mybir.AluOpType.bypass,
                replica_groups=RG,
                ins=[h_ag_in[:]],
                outs=[h_full[:]],
            )

            def gather_block(b, table, width, tag):
                # Two dma_gathers (lo/hi half-tables); real per-core counts
                # come from registers so pad descriptors are skipped.
                msg = sb.tile([P, T * width], bf16, tag=tag, bufs=4)
                if SKIP_GATHER:
                    nc.vector.memset(msg[:, 0:1], 0)
                    return msg
                base = b * T * P // 16
                cl = int(blk_max_lo[b])
                clr = (cl + P - 1) // P * P
                nc.gpsimd.dma_gather(
                    out_ap=msg[:, 0:clr // P * width].rearrange(
                        "p (t d) -> p t d", d=width),
                    in_ap=table[0:half, :],
                    idxs_ap=idx_sb[:, base:base + cl // 16],
                    num_idxs=cl,
                    num_idxs_reg=cl,
                    elem_size=width,
                    single_packet=False,
                )
                ch = int(blk_max_hi[b])
                chr_ = (ch + P - 1) // P * P
                nc.gpsimd.dma_gather(
                    out_ap=msg[:, Tlo * width:Tlo * width + chr_ // P * width]
                    .rearrange("p (t d) -> p t d", d=width),
                    in_ap=table[half:n_pad, :],
                    idxs_ap=idx_sb[
                        :, base + Tlo * P // 16:base + Tlo * P // 16 + ch // 16],
                    num_idxs=ch,
                    num_idxs_reg=ch,
                    elem_size=width,
                    single_packet=False,
                )
                return msg

            def build_selector_chunk(ch):
                # Static selector block streamed from DRAM.
                g0 = ch * C * T * P
                sel = wide.tile([P, C * T * P], bf16, tag="sel")
                if SKIP_SEL:
                    nc.vector.memset(sel[:, 0:1], 0)
                    return sel
                nc.sync.dma_start(
                    out=sel[:], in_=sel_d[:, g0:g0 + C * T * P]
                )
                return sel

            def diag_chunk(ch):
                # Static diag(dinv^2) blocks streamed from DRAM.
                dg = wide.tile([P, C * P], bf16, tag="diag")
                nc.sync.dma_start(
                    out=dg[:], in_=diag_d[:, ch * C * P:(ch + 1) * C * P]
                )
                return dg

            # ---------------- Phase 2: Agg1 + relu + GEMM2 ----------------
            for chk in range(nchunk):
                sel = build_selector_chunk(chk)
                dgw = diag_chunk(chk)
                hsw = wide.tile([P, C * hidden], bf16, tag="hsw")
                nc.sync.dma_start(
                    out=hsw[:].rearrange("p (c d) -> p c d", d=hidden),
                    in_=h_ag_in[chk * C * P:(chk + 1) * C * P, :]
                    .rearrange("(c p) d -> p c d", p=P),
                )
                h2w = wide.tile([P, C * ncls_pad], bf16, tag="h2w")
                for j in range(C):
                    b = chk * C + j
                    msg = gather_block(b, h_full, hidden, "msg")
                    psum1 = psum.tile([P, P], f32, tag="psum1")
                    nt_mm = T if not SKIP_MM else 1
                    for t in range(nt_mm):
                        nc.tensor.matmul(
                            out=psum1[:],
                            lhsT=msg[:, t * hidden:(t + 1) * hidden],
                            rhs=sel[:, (j * T + t) * P:(j * T + t + 1) * P],
                            start=(t == 0),
                            stop=False,
                        )
                    nc.tensor.matmul(
                        out=psum1[:],
                        lhsT=hsw[:, j * hidden:(j + 1) * hidden],
                        rhs=dgw[:, j * P:(j + 1) * P],
                        start=False, stop=True,
                    )
                    a1 = sb.tile([P, P], bf16, tag="a1")
                    nc.scalar.activation(
                        out=a1[:], in_=psum1[:],
                        func=mybir.ActivationFunctionType.Relu,
                        bias=b1_sb[:, 0:1],
                    )
                    psum2 = psum.tile([P, ncls_pad], f32, tag="psum2")
                    nc.tensor.matmul(
                        out=psum2[:], lhsT=a1[:], rhs=w2_sb[:],
                        start=True, stop=True,
                    )
                    nc.vector.tensor_copy(
                        out=h2w[:, j * ncls_pad:(j + 1) * ncls_pad],
                        in_=psum2[:],
                    )
                nc.sync.dma_start(
                    out=h2_ag_in[chk * C * P:(chk + 1) * C * P, :]
                    .rearrange("(c p) d -> p c d", p=P),
                    in_=h2w[:].rearrange("p (c d) -> p c d", d=ncls_pad),
                )

            # ---------------- AllGather h2 ----------------
            nc.gpsimd.collective_compute(
                "AllGather",
                mybir.AluOpType.bypass,
                replica_groups=RG,
                ins=[h2_ag_in[:]],
                outs=[h2_full[:]],
            )

            # ---------------- Phase 3: Agg2 + bias + log_softmax ----------
            for chk in range(nchunk):
                sel = build_selector_chunk(chk)
                dgw = diag_chunk(chk)
                h2sw = wide.tile([P, C * ncls_pad], bf16, tag="h2sw")
                nc.sync.dma_start(
                    out=h2sw[:].rearrange("p (c d) -> p c d", d=ncls_pad),
                    in_=h2_ag_in[chk * C * P:(chk + 1) * C * P, :]
                    .rearrange("(c p) d -> p c d", p=P),
                )
                logw = wide.tile([P, C * N_CLASSES], f32, tag="logw")
                for j in range(C):
                    b = chk * C + j
                    msg2 = gather_block(b, h2_full, ncls_pad, "msg2")
                    psum_o = psum.tile([P, ncls_pad], f32, tag="psum_o")
                    nt_mm = T if not SKIP_MM else 1
                    for t in range(nt_mm):
                        nc.tensor.matmul(
                            out=psum_o[:],
                            lhsT=sel[:, (j * T + t) * P:(j * T + t + 1) * P],
                            rhs=msg2[:, t * ncls_pad:(t + 1) * ncls_pad],
                            start=(t == 0),
                            stop=False,
                        )
                    nc.tensor.matmul(
                        out=psum_o[:],
                        lhsT=dgw[:, j * P:(j + 1) * P],
                        rhs=h2sw[:, j * ncls_pad:(j + 1) * ncls_pad],
                        start=False, stop=True,
                    )
                    nc.vector.tensor_tensor(
                        out=logw[:, j * N_CLASSES:(j + 1) * N_CLASSES],
                        in0=psum_o[:, 0:N_CLASSES],
                        in1=b2_sb[:], op=mybir.AluOpType.add,
                    )
                # Batched log_softmax over the C blocks.
                lw3 = logw[:].rearrange("p (c d) -> p c d", d=N_CLASSES)
                negm = sb.tile([P, C], f32, tag="negm")
                nc.vector.reduce_max(
                    out=negm[:], in_=lw3, axis=mybir.AxisListType.X
                )
                nc.vector.tensor_scalar_mul(
                    out=negm[:], in0=negm[:], scalar1=-1.0
                )
                lm = wide.tile([P, C * N_CLASSES], f32, tag="lm")
                lm3 = lm[:].rearrange("p (c d) -> p c d", d=N_CLASSES)
                nc.vector.tensor_tensor(
                    out=lm3, in0=lw3,
                    in1=negm[:].to_broadcast([P, C, N_CLASSES]),
                    op=mybir.AluOpType.add,
                )
                expv = wide.tile([P, C * N_CLASSES], f32, tag="expv")
                nc.scalar.activation(
                    out=expv[:], in_=lm[:],
                    func=mybir.ActivationFunctionType.Exp,
                )
                ssum = sb.tile([P, C], f32, tag="ssum")
                nc.vector.reduce_sum(
                    out=ssum[:],
                    in_=expv[:].rearrange("p (c d) -> p c d", d=N_CLASSES),
                    axis=mybir.AxisListType.X,
                )
                lns = sb.tile([P, C], f32, tag="lns")
                nc.scalar.activation(
                    out=lns[:], in_=ssum[:],
                    func=mybir.ActivationFunctionType.Ln,
                )
                outt = wide.tile([P, C * N_CLASSES], f32, tag="outt")
                nc.vector.tensor_tensor(
                    out=outt[:].rearrange("p (c d) -> p c d", d=N_CLASSES),
                    in0=lm3,
                    in1=lns[:].to_broadcast([P, C, N_CLASSES]),
                    op=mybir.AluOpType.subtract,
                )
                nc.sync.dma_start(
                    out=out_d[chk * C * P:(chk + 1) * C * P, :]
                    .rearrange("(c p) d -> p c d", p=P),
                    in_=outt[:].rearrange("p (c d) -> p c d", d=N_CLASSES),
                )

    nc.compile()
    return nc


# --------------------------------------------------------------------------
# Host orchestration
# --------------------------------------------------------------------------

def _run(x, edge_index, W1, b1, W2, b2, blocks_per_core):
    from concourse.bass_utils import run_bass_kernel_spmd

    global LAST_RESULT

    x = np.asarray(x, dtype=np.float32)
    W1 = np.asarray(W1, dtype=np.float32)
    b1v = np.asarray(b1, dtype=np.float32).reshape(-1)
    W2 = np.asarray(W2, dtype=np.float32)
    b2v = np.asarray(b2, dtype=np.float32).reshape(-1)

    n_nodes, f_in = x.shape
    hidden = W1.shape[1]
    ncls = W2.shape[1]
    ncls_pad = P
    assert hidden == P and ncls == N_CLASSES

    shard = blocks_per_core * P
    n_pad = NCORES * shard
    assert n_pad >= n_nodes

    (idx16, counts, sel_tab, diag_tab, Tlo, Thi, perm,
     blk_max_lo, blk_max_hi) = _preprocess(
        edge_index, n_nodes, blocks_per_core
    )
    T = Tlo + Thi

    nc = _build_program(f_in, hidden, ncls_pad, blocks_per_core, Tlo, Thi,
                        blk_max_lo, blk_max_hi)

    kt = f_in // P
    bf = ml_dtypes.bfloat16

    # Permuted, padded x: row s holds x[perm[s]].
    x_pad = np.zeros((n_pad, f_in), np.float32)
    valid = perm >= 0
    x_pad[valid] = x[perm[valid]]
    w1r = np.ascontiguousarray(
        W1.reshape(kt, P, hidden).transpose(1, 0, 2).reshape(P, kt * hidden)
    ).astype(bf)
    w2p = np.zeros((hidden, ncls_pad), np.float32)
    w2p[:, :ncls] = W2
    b2t = np.ascontiguousarray(
        np.broadcast_to(b2v[None, :], (P, N_CLASSES))
    ).astype(np.float32)

    in_maps = []
    for c in range(NCORES):
        xt_c = np.ascontiguousarray(
            x_pad[c * shard:(c + 1) * shard].T
        ).astype(bf)
        w = np.ascontiguousarray(idx16[c].reshape(-1, 16).T)
        idx_wrapped = np.ascontiguousarray(np.tile(w, (8, 1)))
        in_maps.append({
            "xt": xt_c,
            "w1": w1r,
            "b1": b1v.reshape(P, 1).copy(),
            "w2": w2p.astype(bf),
            "b2t": b2t,
            "idx16": idx_wrapped,
            "selt": np.ascontiguousarray(sel_tab[c]),
            "diagt": np.ascontiguousarray(diag_tab[c]),
        })

    res = run_bass_kernel_spmd(
        nc, in_maps, core_ids=list(range(NCORES)),
        trace=TRACE, trace_kwargs=dict(TRACE_KWARGS),
    )
    LAST_RESULT = {
        "exec_time_ns": res.exec_time_ns,
        "mean_exec_time_ns": res.mean_exec_time_ns,
        "instructions_and_trace": res.instructions_and_trace,
        "profile_json": res.profile_json,
        "T": T,
        "Tlo": Tlo,
        "Thi": Thi,
        "nc": nc,
        "in_maps": in_maps,
        "perm": perm,
    }
    out_pad = np.concatenate([r["out"] for r in res.results], axis=0)
    out = np.zeros((n_nodes, N_CLASSES), np.float32)
    out[perm[valid]] = out_pad[valid]
    return out


def unpermute(out_pad_concat, perm, n_nodes):
    valid = perm >= 0
    out = np.zeros((n_nodes, N_CLASSES), np.float32)
    out[perm[valid]] = out_pad_concat[valid]
    return out


def kernel(x, edge_index, W1, b1, W2, b2):
    n_nodes = np.asarray(x).shape[0]
    blocks_per_core = int(math.ceil(n_nodes / (NCORES * P)))
    return _run(x, edge_index, W1, b1, W2, b2, blocks_per_core)
